# revision 1
# baseline (speedup 1.0000x reference)
"""Trainium2 Bass kernel for BDLSAGE GNN message passing (8 NeuronCores).

Strategy:
  - Nodes sharded across 8 cores by destination id (12500 real + 44 pad rows
    per core). Hidden state h is stored normalized (h_hat = deg^-1/2 * h) in
    fp16 so the per-edge norm multiply disappears from the hop loop.
  - Each hop: per-edge gather of h_hat rows via gpsimd.dma_gather (int16
    indices into four <=32k-row chunks of the replicated table), segment-sum
    via TensorE one-hot matmuls (S built on-chip with is_equal vs an iota
    constant; PSUM fp32 accumulate), per-node rescale by deg^-1 (a^2), then an
    8-core AllGather of the fp16 shard rebuilds the full table.
  - Snapshot hops (1,2,5,20) accumulate softmax(attention)-weighted copies
    into an fp32 rho buffer; the FFN + inverse bundle transform run in fp32.
"""

import numpy as np

N = 100000
DIM = 128
NC = 8
SR = 12500          # real nodes per core shard
NW = 98             # 128-dst windows per core
SH = NW * 128       # padded shard size = 12544
NPAD = SH * NC      # padded global table = 100352
CHUNK = NPAD // 4   # int16-addressable gather window = 25088
GRP = 4             # dst windows per gather call
SNAPS = {1: 0, 2: 1, 5: 2, 20: 3}
HOPS = int(__import__("os").environ.get("KHOPS", "20"))
_SKIP_GATHER = __import__("os").environ.get("KSKIP_GATHER", "") == "1"
_SKIP_COLL = __import__("os").environ.get("KSKIP_COLL", "") == "1"
PAD_COL = 999.0     # column id that matches no iota slot -> zero S row
PAD_IDX = SR        # local row 12500 of each chunk is an all-zero pad row

GROUPS = [list(range(g, min(g + GRP, NW))) for g in range(0, NW, GRP)]
# gather calls span pairs of psum groups (8 windows) to halve SWDGE overhead
CGRP = 8
CGROUPS = [list(range(g, min(g + CGRP, NW))) for g in range(0, NW, CGRP)]


def _softmax(v):
    e = np.exp(v - v.max())
    return (e / e.sum()).astype(np.float64)


def _prep(inputs):
    x = np.ascontiguousarray(np.asarray(inputs["x"], np.float32))
    nr = np.asarray(inputs["node_rep"], np.float32).reshape(N, DIM)
    src = np.asarray(inputs["src"], np.int64)
    dst = np.asarray(inputs["dst"], np.int64)
    att = _softmax(np.asarray(inputs["attention"], np.float64))
    W1 = np.asarray(inputs["W1"], np.float32)
    b1 = np.asarray(inputs["b1"], np.float32)
    W2 = np.asarray(inputs["W2"], np.float32)
    b2 = np.asarray(inputs["b2"], np.float32)

    deg = np.bincount(src, minlength=N).astype(np.float64)
    a = np.where(deg > 0, deg ** -0.5, 0.0).astype(np.float32)
    a2 = (a * a).astype(np.float32)

    c_of = dst // SR
    rem_d = dst % SR

    # Source ids are independent of the dst->slot permutation below: a
    # node's gather row is (src//SR)*SH + perm_src... but perm changes the
    # row layout too, so src indexing must use the same per-core perm.
    # We therefore compute gather indices after the permutation.

    # --- per-core balanced assignment of nodes to (window, slot) ---
    # Greedy vector bin-packing: equalize per-(window, chunk) edge counts so
    # nearly every cell needs exactly ceil(512/128)=4 tiles. Cells of the
    # first BIGW windows get a 640 cap so overflow concentrates in shared
    # cells across cores (keeps the global max-T structure small).
    BIGW = 8
    k_src_raw = ((src // SR) * SH + (src % SR)) // CHUNK  # pre-perm chunk id

    perms = []          # per core: rem -> n_local
    for c in range(NC):
        sel_c = np.nonzero(c_of == c)[0]
        dv = np.zeros((SR, 4), np.int32)
        np.add.at(dv, (rem_d[sel_c], k_src_raw[sel_c]), 1)
        order = np.argsort(-dv.sum(1), kind="stable")
        L = np.zeros((NW, 4), np.float64)
        C = np.zeros(NW, np.int64)
        cap = np.full((NW, 4), 512.0)
        cap[:BIGW, :] = 640.0
        n_local = np.empty(SR, np.int64)
        for v in order:
            feas = np.nonzero(C < 128)[0]
            Lf = L[feas] + dv[v]
            score = (np.maximum(Lf - cap[feas], 0).sum(1) * 1e9
                     + (Lf * Lf).sum(1))
            w = feas[np.argmin(score)]
            n_local[v] = C[w] * NW + w
            C[w] += 1
            L[w] += dv[v]
        perms.append(n_local)

    # chunk ids depend on src-side permutation of the src's own core
    g_src = np.empty(len(src), np.int64)
    for c in range(NC):
        m = src // SR == c
        g_src[m] = c * SH + perms[c][src[m] % SR]
    k_e = g_src // CHUNK
    idx_e = (g_src % CHUNK).astype(np.int16)

    w_e = np.empty(len(dst), np.int64)
    col_e = np.empty(len(dst), np.int64)
    for c in range(NC):
        m = c_of == c
        nl = perms[c][rem_d[m]]
        w_e[m] = nl % NW
        col_e[m] = nl // NW

    # per-core sorted cell layout
    cores = []
    cnts = np.zeros((NC, NW, 4), np.int64)
    for c in range(NC):
        sel = np.nonzero(c_of == c)[0]
        key = w_e[sel] * 4 + k_e[sel]
        order = np.argsort(key, kind="stable")
        sel = sel[order]
        cnt = np.bincount(w_e[sel] * 4 + k_e[sel], minlength=NW * 4).reshape(NW, 4)
        cnts[c] = cnt
        cores.append((sel, cnt))

    T = np.maximum(np.ceil(cnts.max(axis=0) / 128).astype(np.int64), 1)  # [NW,4]
    TT = int(T.sum())
    if __import__("os").environ.get("KVERBOSE"):
        print(f"[prep] TT={TT} tiles, inflation={TT * 128 * NC / len(src):.3f}")

    # program tile order: for call-grp g8: for chunk k: for w in g8: T[w,k]
    tile_base = np.zeros((NW, 4), np.int64)
    call_off = {}       # (cgi,k) -> idx column offset (in int16 cols of 16)
    call_n = {}         # (cgi,k) -> num idxs
    t_run = 0
    col_run = 0
    for cgi, cgrp in enumerate(CGROUPS):
        for k in range(4):
            n_call = int(T[cgrp, k].sum()) * 128
            call_off[(cgi, k)] = col_run
            call_n[(cgi, k)] = n_call
            for w in cgrp:
                tile_base[w, k] = t_run
                t_run += int(T[w, k])
            col_run += n_call // 16
    IDX_COLS = col_run
    assert t_run == TT

    per_core = []
    for c in range(NC):
        sel, cnt = cores[c]
        starts = np.concatenate([[0], np.cumsum(cnt.reshape(-1))])
        idx_flat = np.full(TT * 128, PAD_IDX, np.int16)
        col_flat = np.full(TT * 128, PAD_COL, np.float32)
        for w in range(NW):
            for k in range(4):
                cell = w * 4 + k
                n = int(cnt[w, k])
                if n == 0:
                    continue
                e = sel[starts[cell]:starts[cell] + n]
                # ascending src order within the cell: consecutive gather
                # descriptors hit nearby HBM addresses (row-buffer locality)
                e = e[np.argsort(idx_e[e], kind="stable")]
                off = int(tile_base[w, k]) * 128
                idx_flat[off:off + n] = idx_e[e]
                col_flat[off:off + n] = col_e[e].astype(np.float32)

        idx_arr = np.zeros((128, IDX_COLS), np.int16)
        for cgi, cgrp in enumerate(CGROUPS):
            for k in range(4):
                o = call_off[(cgi, k)]
                n = call_n[(cgi, k)]
                fo = int(tile_base[cgrp[0], k]) * 128
                block = idx_flat[fo:fo + n].reshape(-1, 16).T  # [16, n/16]
                idx_arr[:, o:o + n // 16] = np.tile(block, (8, 1))

        colid = np.ascontiguousarray(col_flat.reshape(TT, 128).T)  # [128, TT]

        # node-ordered shard arrays (row n_local = perms[c][rem] = p*98 + w)
        pc = perms[c]

        def shardify(full, width):
            arr = np.zeros((SH, width), np.float32)
            arr[pc] = full[c * SR:(c + 1) * SR]
            return np.ascontiguousarray(arr.reshape(128, NW * width))

        x_c = shardify(x, DIM)
        nr_c = shardify(nr, DIM)
        a_pad = np.zeros(SH, np.float32); a_pad[pc] = a[c * SR:(c + 1) * SR]
        a2_pad = np.zeros(SH, np.float32); a2_pad[pc] = a2[c * SR:(c + 1) * SR]
        per_core.append(dict(
            idx=idx_arr, colid=colid, x=x_c, nr=nr_c, perm=pc,
            a=np.ascontiguousarray(a_pad.reshape(128, NW)),
            a2=np.ascontiguousarray(a2_pad.reshape(128, NW)),
        ))

    consts = dict(
        iota=np.tile(np.arange(128, dtype=np.float16), (128, 1)),
        ident=np.eye(128, dtype=np.float32),
        w1=np.ascontiguousarray(W1),                                   # [128,256]
        w2=np.ascontiguousarray(np.concatenate([W2[0:128], W2[128:256]], axis=1)),  # [128,256]
        b1=np.ascontiguousarray(b1.reshape(2, 128).T),                 # [128,2]
        b2=np.ascontiguousarray(b2.reshape(128, 1)),                   # [128,1]
    )
    struct = dict(T=T, TT=TT, IDX_COLS=IDX_COLS, call_off=call_off,
                  call_n=call_n, tile_base=tile_base, att=att)
    return per_core, consts, struct


def _build(struct):
    import concourse.bacc as bacc
    import concourse.bass as bass
    import concourse.mybir as mybir
    import concourse.tile as tile
    from concourse.library_config import mlp

    f32 = mybir.dt.float32
    f16 = mybir.dt.float16
    i16 = mybir.dt.int16
    EQ = mybir.AluOpType.is_equal
    MUL = mybir.AluOpType.mult
    ADD = mybir.AluOpType.add

    T = struct["T"]; TT = struct["TT"]; IDX_COLS = struct["IDX_COLS"]
    call_off = struct["call_off"]; call_n = struct["call_n"]
    tile_base = struct["tile_base"]; att = struct["att"]

    nc = bacc.Bacc("TRN2", target_bir_lowering=False, debug=False,
                   num_devices=NC)

    x_in = nc.dram_tensor("x_in", [128, NW * DIM], f32, kind="ExternalInput")
    nr_in = nc.dram_tensor("nr_in", [128, NW * DIM], f32, kind="ExternalInput")
    idx_in = nc.dram_tensor("idx_in", [128, IDX_COLS], i16, kind="ExternalInput")
    colid_in = nc.dram_tensor("colid_in", [128, TT], f32, kind="ExternalInput")
    a_in = nc.dram_tensor("a_in", [128, NW], f32, kind="ExternalInput")
    a2_in = nc.dram_tensor("a2_in", [128, NW], f32, kind="ExternalInput")
    iota_in = nc.dram_tensor("iota_in", [128, 128], f16, kind="ExternalInput")
    ident_in = nc.dram_tensor("ident_in", [128, 128], f32, kind="ExternalInput")
    w1_in = nc.dram_tensor("w1_in", [128, 256], f32, kind="ExternalInput")
    w2_in = nc.dram_tensor("w2_in", [128, 256], f32, kind="ExternalInput")
    b1_in = nc.dram_tensor("b1_in", [128, 2], f32, kind="ExternalInput")
    b2_in = nc.dram_tensor("b2_in", [128, 1], f32, kind="ExternalInput")
    out_msg = nc.dram_tensor("out_msg", [128, NW * DIM], f32, kind="ExternalOutput")

    with tile.TileContext(nc) as tc:
        nc.gpsimd.load_library(mlp)
        with (
            tc.tile_pool(name="const", bufs=1) as cpool,
            tc.tile_pool(name="dram", bufs=1, space="DRAM") as dpool,
            tc.tile_pool(name="msg", bufs=4) as mpool,
            tc.tile_pool(name="s", bufs=48) as spool,
            tc.tile_pool(name="io", bufs=4) as iopool,
            tc.tile_pool(name="work", bufs=2) as wpool,
        ):
            # persistent SBUF state
            idx_sb = cpool.tile([128, IDX_COLS], i16, tag="idx")
            colid_sb = cpool.tile([128, TT], f32, tag="colid")
            a_sb = cpool.tile([128, NW], f32, tag="a")
            a2_sb = cpool.tile([128, NW], f32, tag="a2")
            iota_sb = cpool.tile([128, 128], f16, tag="iota")
            ident_sb = cpool.tile([128, 128], f32, tag="ident")
            w1_sb = cpool.tile([128, 256], f32, tag="w1")
            w2_sb = cpool.tile([128, 256], f32, tag="w2")
            b1_sb = cpool.tile([128, 2], f32, tag="b1")
            b2_sb = cpool.tile([128, 1], f32, tag="b2")
            stage = cpool.tile([128, NW, 128], f16, tag="stage")
            rho = cpool.tile([128, NW, 128], f32, tag="rho")

            for sb, dr in [(idx_sb, idx_in), (colid_sb, colid_in),
                           (a_sb, a_in), (a2_sb, a2_in), (iota_sb, iota_in),
                           (ident_sb, ident_in), (w1_sb, w1_in), (w2_sb, w2_in),
                           (b1_sb, b1_in), (b2_sb, b2_in)]:
                nc.sync.dma_start(sb[:], dr[:])

            nc.vector.memset(rho[:], 0.0)

            cc_in = []
            cc_out = []
            for i in range(HOPS):
                cc_in.append(dpool.tile([SH, DIM], f16, tag=f"cc_in{i % 2}",
                                        name=f"cc_in{i}"))
                cc_out.append(dpool.tile([NPAD, DIM], f16, tag=f"cc_out{i}",
                                         name=f"cc_out{i}",
                                         addr_space="Shared"))

            def bundle_ops(out_ap, nr_ap, in_ap, tmp_ap, inverse):
                """out[b,c,e] = sum_d nr[b, c, d or (d,c)] * in[b,d,e] on DVE."""
                nr4 = nr_ap.rearrange("p (b c d) -> p b c d", b=8, c=4, d=4)
                in4 = in_ap.rearrange("p (b d e) -> p b d e", b=8, d=4, e=4)
                out4 = out_ap.rearrange("p (b c e) -> p b c e", b=8, c=4, e=4)
                tmp4 = tmp_ap.rearrange("p (b c e) -> p b c e", b=8, c=4, e=4)
                for d in range(4):
                    if inverse:
                        nr_d = nr4[:, :, d:d + 1, :].rearrange("p b o c -> p b (o c)")
                        nr_b = nr_d.unsqueeze(3).broadcast_to((128, 8, 4, 4))
                    else:
                        nr_b = nr4[:, :, :, d:d + 1].broadcast_to((128, 8, 4, 4))
                    in_b = in4[:, :, d:d + 1, :].broadcast_to((128, 8, 4, 4))
                    tgt = out4 if d == 0 else tmp4
                    nc.any.tensor_tensor(tgt, nr_b, in_b, MUL)
                    if d > 0:
                        nc.any.tensor_tensor(out4, out4, tmp4, ADD)

            # ---- pre-stage: h0 = bundle(nr, x); stage = a * h0 (fp16) ----
            for w in range(NW):
                x_t = iopool.tile([128, 128], f32, tag="xt")
                nr_t = iopool.tile([128, 128], f32, tag="nrt")
                nc.sync.dma_start(x_t[:], x_in[:, w * 128:(w + 1) * 128])
                nc.sync.dma_start(nr_t[:], nr_in[:, w * 128:(w + 1) * 128])
                h0 = wpool.tile([128, 128], f32, tag="h0")
                tmp = wpool.tile([128, 128], f32, tag="tmpb")
                bundle_ops(h0[:], nr_t[:], x_t[:], tmp[:], inverse=False)
                nc.any.tensor_scalar(stage[:, w, :], h0[:], a_sb[:, w:w + 1],
                                     None, MUL)
            nc.sync.dma_start(
                cc_in[0][:].rearrange("(p w) e -> p w e", p=128, w=NW), stage[:])
            if _SKIP_COLL:
                nc.sync.dma_start(cc_out[0][0:SH, :], cc_in[0][:])
            else:
                nc.gpsimd.collective_compute(
                    "AllGather", mybir.AluOpType.bypass,
                    replica_groups=[list(range(NC))],
                    ins=[cc_in[0][:]], outs=[cc_out[0][:]])

            # ---- hop loop ----
            with tc.tile_pool(name="psum", bufs=2, space="PSUM") as pspool:
                for t in range(1, HOPS + 1):
                    prev = cc_out[t - 1]
                    for cgi, cgrp in enumerate(CGROUPS):
                        msgs = {}
                        for k in range(4):
                            n = call_n[(cgi, k)]
                            o = call_off[(cgi, k)]
                            ntile = n // 128
                            msg = mpool.tile([128, ntile, 128], f16, tag="msg",
                                             name="msg")
                            msgs[k] = msg
                            if _SKIP_GATHER:
                                nc.vector.memset(msg[:], 0.0)
                            else:
                                nc.gpsimd.dma_gather(
                                    msg[:], prev[k * CHUNK:(k + 1) * CHUNK, :],
                                    idx_sb[:, o:o + n // 16], n, n, 128,
                                    single_packet=False)
                        for sub in range(0, len(cgrp), GRP):
                            grp = cgrp[sub:sub + GRP]
                            psums = {w: pspool.tile([128, 128], f32,
                                                    tag=f"ps{i}", name=f"ps{i}")
                                     for i, w in enumerate(grp)}
                            # build every S tile of the sub-group before its
                            # matmuls so DVE/ACT pacing never gates PE
                            s_tiles = {}
                            for k in range(4):
                                for w in grp:
                                    for j in range(int(T[w, k])):
                                        tix = int(tile_base[w, k]) + j
                                        s_t = spool.tile([128, 128], f16,
                                                         tag="s", name="s_t")
                                        nc.any.tensor_scalar(
                                            s_t[:], iota_sb[:],
                                            colid_sb[:, tix:tix + 1], None, EQ)
                                        s_tiles[(k, w, j)] = s_t
                            for k in range(4):
                                for w in grp:
                                    for j in range(int(T[w, k])):
                                        tix = int(tile_base[w, k]) + j
                                        jj = tix - int(tile_base[cgrp[0], k])
                                        nc.tensor.matmul(
                                            psums[w][:], s_tiles[(k, w, j)][:],
                                            msgs[k][:, jj, :],
                                            start=(k == 0 and j == 0),
                                            stop=(k == 3
                                                  and j == int(T[w, 3]) - 1))
                            for w in grp:
                                if t in SNAPS:
                                    tmp = wpool.tile([128, 128], f32,
                                                     tag="snap")
                                    nc.any.tensor_scalar(tmp[:], psums[w][:],
                                                         float(att[SNAPS[t]]),
                                                         None, MUL)
                                    nc.any.tensor_tensor(rho[:, w, :],
                                                         rho[:, w, :],
                                                         tmp[:], ADD)
                                if t < HOPS:
                                    nc.vector.tensor_scalar(stage[:, w, :],
                                                            psums[w][:],
                                                            a2_sb[:, w:w + 1],
                                                            None, MUL)
                            if t < HOPS:
                                g0 = grp[0]
                                cc_v = cc_in[t][:].rearrange(
                                    "(p w) e -> p w e", p=128, w=NW)
                                nc.sync.dma_start(
                                    cc_v[:, g0:g0 + len(grp), :],
                                    stage[:, g0:g0 + len(grp), :])
                    if t < HOPS:
                        if _SKIP_COLL:
                            nc.sync.dma_start(cc_out[t][0:SH, :], cc_in[t][:])
                        else:
                            nc.gpsimd.collective_compute(
                                "AllGather", mybir.AluOpType.bypass,
                                replica_groups=[list(range(NC))],
                                ins=[cc_in[t][:]], outs=[cc_out[t][:]])

            # ---- post: r = a*rho; FFN; inverse bundle; write out ----
            with tc.tile_pool(name="psum2", bufs=1, space="PSUM") as ps2pool:
                for c0 in range(0, NW, 4):
                    ws = list(range(c0, min(c0 + 4, NW)))
                    nwc = len(ws)
                    rT = wpool.tile([128, 4 * 128], f32, tag="rT")
                    for i, w in enumerate(ws):
                        r_t = wpool.tile([128, 128], f32, tag="rt")
                        nc.any.tensor_scalar(r_t[:], rho[:, w, :],
                                             a_sb[:, w:w + 1], None, MUL)
                        p_t = ps2pool.tile([128, 128], f32, tag="ptr")
                        nc.tensor.transpose(p_t[:], r_t[:], ident_sb[:])
                        nc.vector.tensor_copy(rT[:, i * 128:(i + 1) * 128],
                                              p_t[:])
                    nn = nwc * 128
                    pg0 = ps2pool.tile([128, 512], f32, tag="pg0")
                    pg1 = ps2pool.tile([128, 512], f32, tag="pg1")
                    nc.tensor.matmul(pg0[:, :nn], w1_sb[:, 0:128], rT[:, :nn],
                                     start=True, stop=True)
                    nc.tensor.matmul(pg1[:, :nn], w1_sb[:, 128:256], rT[:, :nn],
                                     start=True, stop=True)
                    g0_sb = wpool.tile([128, 4 * 128], f32, tag="g0")
                    g1_sb = wpool.tile([128, 4 * 128], f32, tag="g1")
                    nc.scalar.activation(g0_sb[:, :nn], pg0[:, :nn],
                                         mybir.ActivationFunctionType.Gelu,
                                         bias=b1_sb[:, 0:1])
                    nc.scalar.activation(g1_sb[:, :nn], pg1[:, :nn],
                                         mybir.ActivationFunctionType.Gelu,
                                         bias=b1_sb[:, 1:2])
                    ph = ps2pool.tile([128, 512], f32, tag="ph")
                    nc.tensor.matmul(ph[:, :nn], w2_sb[:, 0:128], g0_sb[:, :nn],
                                     start=True, stop=False)
                    nc.tensor.matmul(ph[:, :nn], w2_sb[:, 128:256],
                                     g1_sb[:, :nn], start=False, stop=True)
                    h2 = wpool.tile([128, 4 * 128], f32, tag="h2")
                    nc.any.tensor_scalar(h2[:, :nn], ph[:, :nn], b2_sb[:, 0:1],
                                         None, ADD)
                    for i, w in enumerate(ws):
                        pb = ps2pool.tile([128, 128], f32, tag="pb")
                        nc.tensor.transpose(pb[:], h2[:, i * 128:(i + 1) * 128],
                                            ident_sb[:])
                        hb = wpool.tile([128, 128], f32, tag="hb")
                        nc.vector.tensor_copy(hb[:], pb[:])
                        nr_t = iopool.tile([128, 128], f32, tag="nrt2")
                        nc.sync.dma_start(nr_t[:],
                                          nr_in[:, w * 128:(w + 1) * 128])
                        mo = wpool.tile([128, 128], f32, tag="mo")
                        tmp = wpool.tile([128, 128], f32, tag="tmpb2")
                        bundle_ops(mo[:], nr_t[:], hb[:], tmp[:], inverse=True)
                        nc.sync.dma_start(out_msg[:, w * 128:(w + 1) * 128],
                                          mo[:])

    nc.compile()
    return nc


LAST_RESULTS = None  # BassKernelResults of the most recent kernel() call
LAST_NC = None
LAST_IN_MAPS = None


def kernel(**inputs) -> np.ndarray:
    from concourse.bass_utils import run_bass_kernel_spmd

    per_core, consts, struct = _prep(inputs)
    nc = _build(struct)

    in_maps = []
    for c in range(NC):
        d = per_core[c]
        in_maps.append({
            "x_in": d["x"], "nr_in": d["nr"], "idx_in": d["idx"],
            "colid_in": d["colid"], "a_in": d["a"], "a2_in": d["a2"],
            "iota_in": consts["iota"], "ident_in": consts["ident"],
            "w1_in": consts["w1"], "w2_in": consts["w2"],
            "b1_in": consts["b1"], "b2_in": consts["b2"],
        })

    res = run_bass_kernel_spmd(nc, in_maps, core_ids=list(range(NC)))
    global LAST_RESULTS, LAST_NC, LAST_IN_MAPS
    LAST_RESULTS = res
    LAST_NC = nc
    LAST_IN_MAPS = in_maps

    x = np.asarray(inputs["x"], np.float32)
    msg = np.empty((N, DIM), np.float32)
    for c in range(NC):
        arr = res.results[c]["out_msg"].reshape(SH, DIM)
        msg[c * SR:(c + 1) * SR] = arr[per_core[c]["perm"]]
    return np.concatenate([x, msg], axis=1)



# revision 7
# speedup vs baseline: 1.1312x; 1.1312x over previous
"""Trainium2 Bass kernel for BDLSAGE GNN message passing (8 NeuronCores).

Strategy:
  - Nodes sharded across 8 cores by destination id (12500 real + 44 pad rows
    per core). Hidden state h is stored normalized (h_hat = deg^-1/2 * h) in
    fp16 so the per-edge norm multiply disappears from the hop loop.
  - Window-major shard layout (table row = window*128 + slot) so the shard
    splits into 4 contiguous "pieces" by window range. Each hop runs FOUR
    piece AllGathers, each issued as soon as its windows' outputs are staged,
    so collectives overlap the remaining windows' compute and the next hop's
    gathers on other chunks (tile data deps keep it correct).
  - Each hop: per-edge gather of h_hat rows via gpsimd.dma_gather (int16
    indices into the 4 piece chunks; 4 SWDGE queues, one per chunk, to keep
    4 descriptors in flight per DMA engine), segment-sum via TensorE one-hot
    matmuls (S built on-chip with is_equal vs an iota constant; PSUM fp32
    accumulate), per-node rescale by deg^-1 (a^2).
  - Snapshot hops (1,2,5,20) accumulate softmax(attention)-weighted copies
    into an fp32 rho buffer; the FFN + inverse bundle transform run in fp32.
"""

import numpy as np

N = 100000
DIM = 128
NC = 8
SR = 12500          # real nodes per core shard
NW = 98             # 128-dst windows per core
SH = NW * 128       # padded shard size = 12544
GRP = 4             # dst windows per psum sub-group
SNAPS = {1: 0, 2: 1, 5: 2, 20: 3}
HOPS = int(__import__("os").environ.get("KHOPS", "20"))
_SKIP_GATHER = __import__("os").environ.get("KSKIP_GATHER", "") == "1"
_SKIP_COLL = __import__("os").environ.get("KSKIP_COLL", "") == "1"
PAD_COL = 999.0     # column id that matches no iota slot -> zero S row

# window ranges of the 4 AllGather pieces (== gather chunks)
PIECES = [(0, 28), (28, 56), (56, 80), (80, 98)]
NKP = len(PIECES)
PIECE_OF_W = np.zeros(NW, np.int64)
for _j, (_w0, _w1) in enumerate(PIECES):
    PIECE_OF_W[_w0:_w1] = _j
PAD_W = [w1 - 1 for (w0, w1) in PIECES]   # slot 127 of these windows is empty

GROUPS = [list(range(g, min(g + GRP, NW))) for g in range(0, NW, GRP)]
# gather calls span pairs of psum groups (8 windows) to halve SWDGE overhead
CGRP = 8
CGROUPS = [list(range(g, min(g + CGRP, NW))) for g in range(0, NW, CGRP)]
# after finishing this cgroup index, issue this AG piece
AG_AT = {3: 0, 6: 1, 9: 2, 12: 3}


def _softmax(v):
    e = np.exp(v - v.max())
    return (e / e.sum()).astype(np.float64)


def _prep_cached(inputs):
    """Disk-cache the expensive edge-structure prep (keyed on src/dst)."""
    import hashlib, pickle, os
    try:
        h = hashlib.sha1()
        h.update(np.asarray(inputs["src"], np.int64).tobytes())
        h.update(np.asarray(inputs["dst"], np.int64).tobytes())
        h.update(b"v3:%d:%d:%s" % (HOPS, GRP, str(PIECES).encode()))
        path = "/tmp/kprep_%s.pkl" % h.hexdigest()[:16]
        if os.path.exists(path):
            with open(path, "rb") as f:
                return pickle.load(f)
    except Exception:
        path = None
    out = _prep(inputs)
    if path is not None:
        try:
            with open(path + ".tmp", "wb") as f:
                pickle.dump(out, f, protocol=4)
            os.replace(path + ".tmp", path)
        except Exception:
            pass
    return out


def _prep(inputs):
    x = np.ascontiguousarray(np.asarray(inputs["x"], np.float32))
    nr = np.asarray(inputs["node_rep"], np.float32).reshape(N, DIM)
    src = np.asarray(inputs["src"], np.int64)
    dst = np.asarray(inputs["dst"], np.int64)
    att = _softmax(np.asarray(inputs["attention"], np.float64))
    W1 = np.asarray(inputs["W1"], np.float32)
    b1 = np.asarray(inputs["b1"], np.float32)
    W2 = np.asarray(inputs["W2"], np.float32)
    b2 = np.asarray(inputs["b2"], np.float32)

    deg = np.bincount(src, minlength=N).astype(np.float64)
    a = np.where(deg > 0, deg ** -0.5, 0.0).astype(np.float32)
    a2 = (a * a).astype(np.float32)

    c_of = dst // SR
    rem_d = dst % SR

    nw_k = np.array([w1 - w0 for (w0, w1) in PIECES], np.int64)
    w0_k = np.array([w0 for (w0, w1) in PIECES], np.int64)
    chunk_rows = nw_k * 128 * NC          # gather chunk size (int16-addressable)
    assert chunk_rows.max() <= 32767 + 1 - 1

    # --- per-core balanced assignment of nodes to (window, slot) ---
    # Greedy vector bin-packing: equalize per-(window, chunk) edge counts.
    # Per-chunk caps track the uneven piece sizes; the first BIGW windows get
    # +128 so overflow concentrates in shared cells across cores.
    BIGW = 8
    exp_cell = 2048.0 * nw_k / NW         # expected edges per (w, k) cell
    base_cap = (np.ceil(exp_cell / 128.0) * 128.0)

    # pre-perm proxy for a src's chunk: identity layout window = rem // 128
    k_src_raw = PIECE_OF_W[np.minimum(src % SR, SH - 1) // 128]

    perms = []          # per core: rem -> n_local = w*128 + slot
    for c in range(NC):
        sel_c = np.nonzero(c_of == c)[0]
        dv = np.zeros((SR, NKP), np.int32)
        np.add.at(dv, (rem_d[sel_c], k_src_raw[sel_c]), 1)
        order = np.argsort(-dv.sum(1), kind="stable")
        L = np.zeros((NW, NKP), np.float64)
        C = np.zeros(NW, np.int64)
        cap = np.tile(base_cap, (NW, 1))
        cap[:BIGW, :] += 128.0
        cmax = np.full(NW, 128, np.int64)
        cmax[PAD_W] = 127                 # reserve an all-zero pad row
        n_local = np.empty(SR, np.int64)
        for v in order:
            feas = np.nonzero(C < cmax)[0]
            Lf = L[feas] + dv[v]
            score = (np.maximum(Lf - cap[feas], 0).sum(1) * 1e9
                     + (Lf * Lf).sum(1))
            w = feas[np.argmin(score)]
            n_local[v] = w * 128 + C[w]
            C[w] += 1
            L[w] += dv[v]
        perms.append(n_local)

    # actual chunk ids/idx from the src's own core permutation
    k_e = np.empty(len(src), np.int64)
    idx_e = np.empty(len(src), np.int64)
    for c in range(NC):
        m = src // SR == c
        nl = perms[c][src[m] % SR]
        ws = nl // 128
        ss = nl % 128
        kk = PIECE_OF_W[ws]
        k_e[m] = kk
        idx_e[m] = c * nw_k[kk] * 128 + (ws - w0_k[kk]) * 128 + ss
    idx_e = idx_e.astype(np.int16)

    w_e = np.empty(len(dst), np.int64)
    col_e = np.empty(len(dst), np.int64)
    for c in range(NC):
        m = c_of == c
        nl = perms[c][rem_d[m]]
        w_e[m] = nl // 128
        col_e[m] = nl % 128

    # per-core sorted cell layout
    cores = []
    cnts = np.zeros((NC, NW, NKP), np.int64)
    for c in range(NC):
        sel = np.nonzero(c_of == c)[0]
        key = w_e[sel] * NKP + k_e[sel]
        order = np.argsort(key, kind="stable")
        sel = sel[order]
        cnt = np.bincount(w_e[sel] * NKP + k_e[sel],
                          minlength=NW * NKP).reshape(NW, NKP)
        cnts[c] = cnt
        cores.append((sel, cnt))

    T = np.maximum(np.ceil(cnts.max(axis=0) / 128).astype(np.int64), 1)
    TT = int(T.sum())
    if __import__("os").environ.get("KVERBOSE"):
        print(f"[prep] TT={TT} tiles, inflation={TT * 128 * NC / len(src):.3f}")

    # program tile order: for call-grp g8: for chunk k: for w in g8: T[w,k]
    tile_base = np.zeros((NW, NKP), np.int64)
    call_off = {}       # (cgi,k) -> idx column offset (in int16 cols of 16)
    call_n = {}         # (cgi,k) -> num idxs
    t_run = 0
    col_run = 0
    for cgi, cgrp in enumerate(CGROUPS):
        for k in range(NKP):
            n_call = int(T[cgrp, k].sum()) * 128
            call_off[(cgi, k)] = col_run
            call_n[(cgi, k)] = n_call
            for w in cgrp:
                tile_base[w, k] = t_run
                t_run += int(T[w, k])
            col_run += n_call // 16
    IDX_COLS = col_run
    assert t_run == TT

    pad_idx_k = nw_k * 128 - 1   # core 0's reserved zero row in each chunk

    per_core = []
    for c in range(NC):
        sel, cnt = cores[c]
        starts = np.concatenate([[0], np.cumsum(cnt.reshape(-1))])
        idx_flat = np.zeros(TT * 128, np.int16)
        col_flat = np.full(TT * 128, PAD_COL, np.float32)
        for w in range(NW):
            for k in range(NKP):
                cell = w * NKP + k
                off = int(tile_base[w, k]) * 128
                ntile = int(T[w, k])
                idx_flat[off:off + ntile * 128] = pad_idx_k[k]
                n = int(cnt[w, k])
                if n == 0:
                    continue
                e = sel[starts[cell]:starts[cell] + n]
                # ascending src order within the cell: consecutive gather
                # descriptors hit nearby HBM addresses (row-buffer locality)
                e = e[np.argsort(idx_e[e], kind="stable")]
                idx_flat[off:off + n] = idx_e[e]
                col_flat[off:off + n] = col_e[e].astype(np.float32)

        idx_arr = np.zeros((128, IDX_COLS), np.int16)
        for cgi, cgrp in enumerate(CGROUPS):
            for k in range(NKP):
                o = call_off[(cgi, k)]
                n = call_n[(cgi, k)]
                fo = int(tile_base[cgrp[0], k]) * 128
                block = idx_flat[fo:fo + n].reshape(-1, 16).T  # [16, n/16]
                idx_arr[:, o:o + n // 16] = np.tile(block, (8, 1))

        colid = np.ascontiguousarray(col_flat.reshape(TT, 128).T)  # [128, TT]

        # node-ordered shard arrays (row n_local = w*128 + slot)
        pc = perms[c]

        def shardify(full, width):
            arr = np.zeros((SH, width), np.float32)
            arr[pc] = full[c * SR:(c + 1) * SR]
            return np.ascontiguousarray(
                arr.reshape(NW, 128, width).transpose(1, 0, 2)
                .reshape(128, NW * width))

        x_c = shardify(x, DIM)
        nr_c = shardify(nr, DIM)
        a_pad = np.zeros(SH, np.float32); a_pad[pc] = a[c * SR:(c + 1) * SR]
        a2_pad = np.zeros(SH, np.float32); a2_pad[pc] = a2[c * SR:(c + 1) * SR]
        per_core.append(dict(
            idx=idx_arr, colid=colid, x=x_c, nr=nr_c, perm=pc,
            a=np.ascontiguousarray(a_pad.reshape(NW, 128).T),
            a2=np.ascontiguousarray(a2_pad.reshape(NW, 128).T),
        ))

    consts = dict(
        iota=np.tile(np.arange(128, dtype=np.float16), (128, 1)),
        ident=np.eye(128, dtype=np.float32),
        w1=np.ascontiguousarray(W1),                                   # [128,256]
        w2=np.ascontiguousarray(np.concatenate([W2[0:128], W2[128:256]], axis=1)),  # [128,256]
        b1=np.ascontiguousarray(b1.reshape(2, 128).T),                 # [128,2]
        b2=np.ascontiguousarray(b2.reshape(128, 1)),                   # [128,1]
    )
    struct = dict(T=T, TT=TT, IDX_COLS=IDX_COLS, call_off=call_off,
                  call_n=call_n, tile_base=tile_base, att=att)
    return per_core, consts, struct


def _build(struct):
    import concourse.bacc as bacc
    import concourse.bass as bass
    import concourse.mybir as mybir
    import concourse.tile as tile
    from concourse.library_config import mlp

    f32 = mybir.dt.float32
    f16 = mybir.dt.float16
    i16 = mybir.dt.int16
    EQ = mybir.AluOpType.is_equal
    MUL = mybir.AluOpType.mult
    ADD = mybir.AluOpType.add

    T = struct["T"]; TT = struct["TT"]; IDX_COLS = struct["IDX_COLS"]
    call_off = struct["call_off"]; call_n = struct["call_n"]
    tile_base = struct["tile_base"]; att = struct["att"]
    nw_k = [w1 - w0 for (w0, w1) in PIECES]

    nc = bacc.Bacc("TRN2", target_bir_lowering=False, debug=False,
                   num_devices=NC, num_swdge_queues=4)

    x_in = nc.dram_tensor("x_in", [128, NW * DIM], f32, kind="ExternalInput")
    nr_in = nc.dram_tensor("nr_in", [128, NW * DIM], f32, kind="ExternalInput")
    idx_in = nc.dram_tensor("idx_in", [128, IDX_COLS], i16, kind="ExternalInput")
    colid_in = nc.dram_tensor("colid_in", [128, TT], f32, kind="ExternalInput")
    a_in = nc.dram_tensor("a_in", [128, NW], f32, kind="ExternalInput")
    a2_in = nc.dram_tensor("a2_in", [128, NW], f32, kind="ExternalInput")
    iota_in = nc.dram_tensor("iota_in", [128, 128], f16, kind="ExternalInput")
    ident_in = nc.dram_tensor("ident_in", [128, 128], f32, kind="ExternalInput")
    w1_in = nc.dram_tensor("w1_in", [128, 256], f32, kind="ExternalInput")
    w2_in = nc.dram_tensor("w2_in", [128, 256], f32, kind="ExternalInput")
    b1_in = nc.dram_tensor("b1_in", [128, 2], f32, kind="ExternalInput")
    b2_in = nc.dram_tensor("b2_in", [128, 1], f32, kind="ExternalInput")
    out_msg = nc.dram_tensor("out_msg", [128, NW * DIM], f32, kind="ExternalOutput")

    with tile.TileContext(nc) as tc:
        nc.gpsimd.load_library(mlp)
        with (
            tc.tile_pool(name="const", bufs=1) as cpool,
            tc.tile_pool(name="dram", bufs=1, space="DRAM") as dpool,
            tc.tile_pool(name="msg", bufs=4) as mpool,
            tc.tile_pool(name="s", bufs=48) as spool,
            tc.tile_pool(name="io", bufs=4) as iopool,
            tc.tile_pool(name="work", bufs=2) as wpool,
        ):
            # persistent SBUF state
            idx_sb = cpool.tile([128, IDX_COLS], i16, tag="idx")
            colid_sb = cpool.tile([128, TT], f32, tag="colid")
            a_sb = cpool.tile([128, NW], f32, tag="a")
            a2_sb = cpool.tile([128, NW], f32, tag="a2")
            iota_sb = cpool.tile([128, 128], f16, tag="iota")
            ident_sb = cpool.tile([128, 128], f32, tag="ident")
            w1_sb = cpool.tile([128, 256], f32, tag="w1")
            w2_sb = cpool.tile([128, 256], f32, tag="w2")
            b1_sb = cpool.tile([128, 2], f32, tag="b1")
            b2_sb = cpool.tile([128, 1], f32, tag="b2")
            stage = cpool.tile([128, NW, 128], f16, tag="stage")
            rho = cpool.tile([128, NW, 128], f32, tag="rho")

            for sb, dr in [(idx_sb, idx_in), (colid_sb, colid_in),
                           (a_sb, a_in), (a2_sb, a2_in), (iota_sb, iota_in),
                           (ident_sb, ident_in), (w1_sb, w1_in), (w2_sb, w2_in),
                           (b1_sb, b1_in), (b2_sb, b2_in)]:
                nc.sync.dma_start(sb[:], dr[:])

            nc.vector.memset(rho[:], 0.0)

            cc_in = []
            cc_out = []
            for i in range(HOPS):
                cc_in.append(dpool.tile([SH, DIM], f16, tag=f"cc_in{i % 2}",
                                        name=f"cc_in{i}"))
                cc_out.append([
                    dpool.tile([NC * nw_k[j] * 128, DIM], f16,
                               tag=f"cc_out{i}_{j}", name=f"cc_out{i}_{j}",
                               addr_space="Shared")
                    for j in range(NKP)])

            def issue_ag(t, j):
                w0, w1 = PIECES[j]
                if _SKIP_COLL:
                    nc.sync.dma_start(cc_out[t][j][0:(w1 - w0) * 128, :],
                                      cc_in[t][w0 * 128:w1 * 128, :])
                else:
                    nc.gpsimd.collective_compute(
                        "AllGather", mybir.AluOpType.bypass,
                        replica_groups=[list(range(NC))],
                        ins=[cc_in[t][w0 * 128:w1 * 128, :]],
                        outs=[cc_out[t][j][:]])

            def bundle_ops(out_ap, nr_ap, in_ap, tmp_ap, inverse):
                """out[b,c,e] = sum_d nr[b, c, d or (d,c)] * in[b,d,e] on DVE."""
                nr4 = nr_ap.rearrange("p (b c d) -> p b c d", b=8, c=4, d=4)
                in4 = in_ap.rearrange("p (b d e) -> p b d e", b=8, d=4, e=4)
                out4 = out_ap.rearrange("p (b c e) -> p b c e", b=8, c=4, e=4)
                tmp4 = tmp_ap.rearrange("p (b c e) -> p b c e", b=8, c=4, e=4)
                for d in range(4):
                    if inverse:
                        nr_d = nr4[:, :, d:d + 1, :].rearrange("p b o c -> p b (o c)")
                        nr_b = nr_d.unsqueeze(3).broadcast_to((128, 8, 4, 4))
                    else:
                        nr_b = nr4[:, :, :, d:d + 1].broadcast_to((128, 8, 4, 4))
                    in_b = in4[:, :, d:d + 1, :].broadcast_to((128, 8, 4, 4))
                    tgt = out4 if d == 0 else tmp4
                    nc.any.tensor_tensor(tgt, nr_b, in_b, MUL)
                    if d > 0:
                        nc.any.tensor_tensor(out4, out4, tmp4, ADD)

            # cc_in viewed as [slot partition, window, feature]
            def cc_in_view(t):
                return cc_in[t][:].rearrange("(w p) e -> p w e", w=NW, p=128)

            # ---- pre-stage: h0 = bundle(nr, x); stage = a * h0 (fp16) ----
            for w in range(NW):
                x_t = iopool.tile([128, 128], f32, tag="xt")
                nr_t = iopool.tile([128, 128], f32, tag="nrt")
                nc.sync.dma_start(x_t[:], x_in[:, w * 128:(w + 1) * 128])
                nc.sync.dma_start(nr_t[:], nr_in[:, w * 128:(w + 1) * 128])
                h0 = wpool.tile([128, 128], f32, tag="h0")
                tmp = wpool.tile([128, 128], f32, tag="tmpb")
                bundle_ops(h0[:], nr_t[:], x_t[:], tmp[:], inverse=False)
                nc.any.tensor_scalar(stage[:, w, :], h0[:], a_sb[:, w:w + 1],
                                     None, MUL)
                nc.sync.dma_start(cc_in_view(0)[:, w:w + 1, :],
                                  stage[:, w:w + 1, :])
                for j in range(NKP):
                    if w == PIECES[j][1] - 1:
                        issue_ag(0, j)

            # ---- hop loop ----
            with tc.tile_pool(name="psum", bufs=2, space="PSUM") as pspool:
                for t in range(1, HOPS + 1):
                    prev = cc_out[t - 1]
                    for cgi, cgrp in enumerate(CGROUPS):
                        msgs = {}
                        for k in range(NKP):
                            n = call_n[(cgi, k)]
                            o = call_off[(cgi, k)]
                            ntile = n // 128
                            msg = mpool.tile([128, ntile, 128], f16, tag="msg",
                                             name="msg")
                            msgs[k] = msg
                            if _SKIP_GATHER:
                                nc.vector.memset(msg[:], 0.0)
                            else:
                                nc.gpsimd.dma_gather(
                                    msg[:], prev[k][:, :],
                                    idx_sb[:, o:o + n // 16], n, n, 128,
                                    single_packet=False, queue_num=k)
                        for sub in range(0, len(cgrp), GRP):
                            grp = cgrp[sub:sub + GRP]
                            psums = {w: pspool.tile([128, 128], f32,
                                                    tag=f"ps{i}", name=f"ps{i}")
                                     for i, w in enumerate(grp)}
                            # build every S tile of the sub-group before its
                            # matmuls so DVE/ACT pacing never gates PE
                            s_tiles = {}
                            for k in range(NKP):
                                for w in grp:
                                    for j in range(int(T[w, k])):
                                        tix = int(tile_base[w, k]) + j
                                        s_t = spool.tile([128, 128], f16,
                                                         tag="s", name="s_t")
                                        nc.any.tensor_scalar(
                                            s_t[:], iota_sb[:],
                                            colid_sb[:, tix:tix + 1], None, EQ)
                                        s_tiles[(k, w, j)] = s_t
                            for k in range(NKP):
                                for w in grp:
                                    for j in range(int(T[w, k])):
                                        tix = int(tile_base[w, k]) + j
                                        jj = tix - int(tile_base[cgrp[0], k])
                                        nc.tensor.matmul(
                                            psums[w][:], s_tiles[(k, w, j)][:],
                                            msgs[k][:, jj, :],
                                            start=(k == 0 and j == 0),
                                            stop=(k == NKP - 1
                                                  and j == int(T[w, NKP - 1]) - 1))
                            for w in grp:
                                if t in SNAPS:
                                    tmp = wpool.tile([128, 128], f32,
                                                     tag="snap")
                                    nc.any.tensor_scalar(tmp[:], psums[w][:],
                                                         float(att[SNAPS[t]]),
                                                         None, MUL)
                                    nc.any.tensor_tensor(rho[:, w, :],
                                                         rho[:, w, :],
                                                         tmp[:], ADD)
                                if t < HOPS:
                                    nc.vector.tensor_scalar(stage[:, w, :],
                                                            psums[w][:],
                                                            a2_sb[:, w:w + 1],
                                                            None, MUL)
                            if t < HOPS:
                                g0 = grp[0]
                                nc.sync.dma_start(
                                    cc_in_view(t)[:, g0:g0 + len(grp), :],
                                    stage[:, g0:g0 + len(grp), :])
                        if t < HOPS and cgi in AG_AT:
                            issue_ag(t, AG_AT[cgi])

            # ---- post: r = a*rho; FFN; inverse bundle; write out ----
            with tc.tile_pool(name="psum2", bufs=1, space="PSUM") as ps2pool:
                for c0 in range(0, NW, 4):
                    ws = list(range(c0, min(c0 + 4, NW)))
                    nwc = len(ws)
                    rT = wpool.tile([128, 4 * 128], f32, tag="rT")
                    for i, w in enumerate(ws):
                        r_t = wpool.tile([128, 128], f32, tag="rt")
                        nc.any.tensor_scalar(r_t[:], rho[:, w, :],
                                             a_sb[:, w:w + 1], None, MUL)
                        p_t = ps2pool.tile([128, 128], f32, tag="ptr")
                        nc.tensor.transpose(p_t[:], r_t[:], ident_sb[:])
                        nc.vector.tensor_copy(rT[:, i * 128:(i + 1) * 128],
                                              p_t[:])
                    nn = nwc * 128
                    pg0 = ps2pool.tile([128, 512], f32, tag="pg0")
                    pg1 = ps2pool.tile([128, 512], f32, tag="pg1")
                    nc.tensor.matmul(pg0[:, :nn], w1_sb[:, 0:128], rT[:, :nn],
                                     start=True, stop=True)
                    nc.tensor.matmul(pg1[:, :nn], w1_sb[:, 128:256], rT[:, :nn],
                                     start=True, stop=True)
                    g0_sb = wpool.tile([128, 4 * 128], f32, tag="g0")
                    g1_sb = wpool.tile([128, 4 * 128], f32, tag="g1")
                    nc.scalar.activation(g0_sb[:, :nn], pg0[:, :nn],
                                         mybir.ActivationFunctionType.Gelu,
                                         bias=b1_sb[:, 0:1])
                    nc.scalar.activation(g1_sb[:, :nn], pg1[:, :nn],
                                         mybir.ActivationFunctionType.Gelu,
                                         bias=b1_sb[:, 1:2])
                    ph = ps2pool.tile([128, 512], f32, tag="ph")
                    nc.tensor.matmul(ph[:, :nn], w2_sb[:, 0:128], g0_sb[:, :nn],
                                     start=True, stop=False)
                    nc.tensor.matmul(ph[:, :nn], w2_sb[:, 128:256],
                                     g1_sb[:, :nn], start=False, stop=True)
                    h2 = wpool.tile([128, 4 * 128], f32, tag="h2")
                    nc.any.tensor_scalar(h2[:, :nn], ph[:, :nn], b2_sb[:, 0:1],
                                         None, ADD)
                    for i, w in enumerate(ws):
                        pb = ps2pool.tile([128, 128], f32, tag="pb")
                        nc.tensor.transpose(pb[:], h2[:, i * 128:(i + 1) * 128],
                                            ident_sb[:])
                        hb = wpool.tile([128, 128], f32, tag="hb")
                        nc.vector.tensor_copy(hb[:], pb[:])
                        nr_t = iopool.tile([128, 128], f32, tag="nrt2")
                        nc.sync.dma_start(nr_t[:],
                                          nr_in[:, w * 128:(w + 1) * 128])
                        mo = wpool.tile([128, 128], f32, tag="mo")
                        tmp = wpool.tile([128, 128], f32, tag="tmpb2")
                        bundle_ops(mo[:], nr_t[:], hb[:], tmp[:], inverse=True)
                        nc.sync.dma_start(out_msg[:, w * 128:(w + 1) * 128],
                                          mo[:])

    nc.compile()
    return nc


LAST_RESULTS = None  # BassKernelResults of the most recent kernel() call
LAST_NC = None
LAST_IN_MAPS = None


def kernel(**inputs) -> np.ndarray:
    from concourse.bass_utils import run_bass_kernel_spmd

    per_core, consts, struct = _prep_cached(inputs)
    nc = _build(struct)

    in_maps = []
    for c in range(NC):
        d = per_core[c]
        in_maps.append({
            "x_in": d["x"], "nr_in": d["nr"], "idx_in": d["idx"],
            "colid_in": d["colid"], "a_in": d["a"], "a2_in": d["a2"],
            "iota_in": consts["iota"], "ident_in": consts["ident"],
            "w1_in": consts["w1"], "w2_in": consts["w2"],
            "b1_in": consts["b1"], "b2_in": consts["b2"],
        })

    trace = __import__("os").environ.get("KTRACE", "") == "1"
    res = run_bass_kernel_spmd(nc, in_maps, core_ids=list(range(NC)),
                               trace=trace)
    global LAST_RESULTS, LAST_NC, LAST_IN_MAPS
    LAST_RESULTS = res
    LAST_NC = nc
    LAST_IN_MAPS = in_maps

    x = np.asarray(inputs["x"], np.float32)
    msg = np.empty((N, DIM), np.float32)
    for c in range(NC):
        arr = res.results[c]["out_msg"].reshape(128, NW, DIM)
        arr = np.ascontiguousarray(arr.transpose(1, 0, 2)).reshape(SH, DIM)
        msg[c * SR:(c + 1) * SR] = arr[per_core[c]["perm"]]
    return np.concatenate([x, msg], axis=1)


# revision 14
# speedup vs baseline: 1.3940x; 1.2324x over previous
"""Trainium2 Bass kernel for BDLSAGE GNN message passing (8 NeuronCores).

Strategy:
  - Nodes sharded across 8 cores by destination id (12500 real + 44 pad rows
    per core). Hidden state h is stored normalized (h_hat = deg^-1/2 * h) in
    fp16 so the per-edge norm multiply disappears from the hop loop.
  - Window-major shard layout (table row = window*128 + slot) so the shard
    splits into 4 contiguous "pieces" by window range. Each hop runs FOUR
    piece AllGathers, each issued as soon as its windows' outputs are staged,
    so collectives overlap the remaining windows' compute and the next hop's
    gathers on other chunks (tile data deps keep it correct).
  - Each hop: per-edge gather of h_hat rows via gpsimd.dma_gather (int16
    indices into the 4 piece chunks; 4 SWDGE queues, one per chunk, to keep
    4 descriptors in flight per DMA engine), segment-sum via TensorE one-hot
    matmuls (S built on-chip with is_equal vs an iota constant; PSUM fp32
    accumulate), per-node rescale by deg^-1 (a^2).
  - Snapshot hops (1,2,5,20) accumulate softmax(attention)-weighted copies
    into an fp32 rho buffer; the FFN + inverse bundle transform run in fp32.
"""

import numpy as np

N = 100000
DIM = 128
NC = 8
SR = 12500          # real nodes per core shard
NW = 98             # 128-dst windows per core
SH = NW * 128       # padded shard size = 12544
GRP = 4             # dst windows per psum sub-group
SNAPS = {1: 0, 2: 1, 5: 2, 20: 3}
HOPS = int(__import__("os").environ.get("KHOPS", "20"))
_SKIP_GATHER = __import__("os").environ.get("KSKIP_GATHER", "") == "1"
_SKIP_COLL = __import__("os").environ.get("KSKIP_COLL", "") == "1"
PAD_COL = 999.0     # column id that matches no iota slot -> zero S row

# window ranges of the 4 AllGather pieces (== gather chunks)
_pw = [int(z) for z in _PIECE_CFG.split(",")]
assert sum(_pw) == NW
PIECES = []
_acc = 0
for _z in _pw:
    PIECES.append((_acc, _acc + _z))
    _acc += _z
NKP = len(PIECES)
assert NKP == 4
PIECE_OF_W = np.zeros(NW, np.int64)
for _j, (_w0, _w1) in enumerate(PIECES):
    PIECE_OF_W[_w0:_w1] = _j
PAD_W = [w1 - 1 for (w0, w1) in PIECES]   # slot 127 of these windows is empty

GROUPS = [list(range(g, min(g + GRP, NW))) for g in range(0, NW, GRP)]
# gather calls span pairs of psum groups (8 windows) to halve SWDGE overhead
CGRP = 8
CGROUPS = [list(range(g, min(g + CGRP, NW))) for g in range(0, NW, CGRP)]
# after finishing cgroup cgi, issue these AG pieces (first cgroup whose
# last window covers the piece end)
AG_AT = {}
for _j, (_w0, _w1) in enumerate(PIECES):
    for _cgi, _cgrp in enumerate(CGROUPS):
        if _cgrp[-1] >= _w1 - 1:
            AG_AT.setdefault(_cgi, []).append(_j)
            break


def _softmax(v):
    e = np.exp(v - v.max())
    return (e / e.sum()).astype(np.float64)


def _prep_cached(inputs):
    """Disk-cache the expensive edge-structure prep (keyed on src/dst)."""
    import hashlib, pickle, os
    try:
        h = hashlib.sha1()
        h.update(np.asarray(inputs["src"], np.int64).tobytes())
        h.update(np.asarray(inputs["dst"], np.int64).tobytes())
        h.update(b"v6:%d:%d:%s" % (HOPS, GRP, str(PIECES).encode()))
        path = "/tmp/kprep_%s.pkl" % h.hexdigest()[:16]
        if os.path.exists(path):
            with open(path, "rb") as f:
                return pickle.load(f)
    except Exception:
        path = None
    out = _prep(inputs)
    if path is not None:
        try:
            with open(path + ".tmp", "wb") as f:
                pickle.dump(out, f, protocol=4)
            os.replace(path + ".tmp", path)
        except Exception:
            pass
    return out


def _prep(inputs):
    x = np.ascontiguousarray(np.asarray(inputs["x"], np.float32))
    nr = np.asarray(inputs["node_rep"], np.float32).reshape(N, DIM)
    src = np.asarray(inputs["src"], np.int64)
    dst = np.asarray(inputs["dst"], np.int64)
    att = _softmax(np.asarray(inputs["attention"], np.float64))
    W1 = np.asarray(inputs["W1"], np.float32)
    b1 = np.asarray(inputs["b1"], np.float32)
    W2 = np.asarray(inputs["W2"], np.float32)
    b2 = np.asarray(inputs["b2"], np.float32)

    deg = np.bincount(src, minlength=N).astype(np.float64)
    a = np.where(deg > 0, deg ** -0.5, 0.0).astype(np.float32)
    a2 = (a * a).astype(np.float32)

    c_of = dst // SR
    rem_d = dst % SR

    nw_k = np.array([w1 - w0 for (w0, w1) in PIECES], np.int64)
    w0_k = np.array([w0 for (w0, w1) in PIECES], np.int64)
    chunk_rows = nw_k * 128 * NC          # gather chunk size (int16-addressable)
    assert chunk_rows.max() <= 32767 + 1 - 1

    # --- per-core balanced assignment of nodes to (window, slot) ---
    # Greedy vector bin-packing: equalize per-(window, chunk) edge counts.
    # Per-chunk caps track the uneven piece sizes; the first BIGW windows get
    # +128 so overflow concentrates in shared cells across cores.
    BIGW = 8
    exp_cell = 2048.0 * nw_k / NW         # expected edges per (w, k) cell
    base_cap = (np.ceil(exp_cell / 128.0) * 128.0)

    # pre-perm proxy for a src's chunk: identity layout window = rem // 128
    k_src_raw = PIECE_OF_W[np.minimum(src % SR, SH - 1) // 128]

    def pack(dv_by_core, piece_lock):
        pms = []
        for c in range(NC):
            dv = dv_by_core[c]
            order = np.argsort(-dv.sum(1), kind="stable")
            L = np.zeros((NW, NKP), np.float64)
            C = np.zeros(NW, np.int64)
            cap = np.tile(base_cap, (NW, 1))
            cap[:BIGW, :] += 128.0
            cmax = np.full(NW, 128, np.int64)
            cmax[PAD_W] = 127             # reserve an all-zero pad row
            n_local = np.empty(SR, np.int64)
            pl = piece_lock[c] if piece_lock is not None else None
            for v in order:
                ok = C < cmax
                if pl is not None:
                    ok &= PIECE_OF_W == pl[v]
                feas = np.nonzero(ok)[0]
                Lf = L[feas] + dv[v]
                score = (np.maximum(Lf - cap[feas], 0).sum(1) * 1e9
                         + (Lf * Lf).sum(1))
                w = feas[np.argmin(score)]
                n_local[v] = w * 128 + C[w]
                C[w] += 1
                L[w] += dv[v]
            pms.append(n_local)
        return pms

    # pass 1: proxy chunk ids -> initial window assignment (fixes each
    # node's PIECE, hence every edge's true chunk id)
    dv1 = []
    for c in range(NC):
        sel_c = np.nonzero(c_of == c)[0]
        dv = np.zeros((SR, NKP), np.int32)
        np.add.at(dv, (rem_d[sel_c], k_src_raw[sel_c]), 1)
        dv1.append(dv)
    perms1 = pack(dv1, None)
    # pass 2: exact chunk ids (pieces frozen) -> balanced repack in-piece
    k_true = np.empty(len(src), np.int64)
    for c in range(NC):
        m = src // SR == c
        k_true[m] = PIECE_OF_W[perms1[c][src[m] % SR] // 128]
    piece_lock = [PIECE_OF_W[perms1[c] // 128] for c in range(NC)]
    dv2 = []
    for c in range(NC):
        sel_c = np.nonzero(c_of == c)[0]
        dv = np.zeros((SR, NKP), np.int32)
        np.add.at(dv, (rem_d[sel_c], k_true[sel_c]), 1)
        dv2.append(dv)
    perms = pack(dv2, piece_lock)

    # actual chunk ids/idx from the src's own core permutation
    k_e = np.empty(len(src), np.int64)
    idx_e = np.empty(len(src), np.int64)
    for c in range(NC):
        m = src // SR == c
        nl = perms[c][src[m] % SR]
        ws = nl // 128
        ss = nl % 128
        kk = PIECE_OF_W[ws]
        k_e[m] = kk
        idx_e[m] = c * nw_k[kk] * 128 + (ws - w0_k[kk]) * 128 + ss
    idx_e = idx_e.astype(np.int16)

    w_e = np.empty(len(dst), np.int64)
    col_e = np.empty(len(dst), np.int64)
    for c in range(NC):
        m = c_of == c
        nl = perms[c][rem_d[m]]
        w_e[m] = nl // 128
        col_e[m] = nl % 128

    # per-core sorted cell layout
    cores = []
    cnts = np.zeros((NC, NW, NKP), np.int64)
    for c in range(NC):
        sel = np.nonzero(c_of == c)[0]
        key = w_e[sel] * NKP + k_e[sel]
        order = np.argsort(key, kind="stable")
        sel = sel[order]
        cnt = np.bincount(w_e[sel] * NKP + k_e[sel],
                          minlength=NW * NKP).reshape(NW, NKP)
        cnts[c] = cnt
        cores.append((sel, cnt))

    T = np.maximum(np.ceil(cnts.max(axis=0) / 128).astype(np.int64), 1)
    TT = int(T.sum())
    if __import__("os").environ.get("KVERBOSE"):
        print(f"[prep] TT={TT} tiles, inflation={TT * 128 * NC / len(src):.3f}")

    # program tile order: for call-grp g8: for chunk k: for w in g8: T[w,k]
    tile_base = np.zeros((NW, NKP), np.int64)
    call_off = {}       # (cgi,k) -> idx column offset (in int16 cols of 16)
    call_n = {}         # (cgi,k) -> num idxs
    t_run = 0
    col_run = 0
    for cgi, cgrp in enumerate(CGROUPS):
        for k in range(NKP):
            n_call = int(T[cgrp, k].sum()) * 128
            call_off[(cgi, k)] = col_run
            call_n[(cgi, k)] = n_call
            for w in cgrp:
                tile_base[w, k] = t_run
                t_run += int(T[w, k])
            col_run += n_call // 16
    IDX_COLS = col_run
    assert t_run == TT

    pad_idx_k = nw_k * 128 - 1   # core 0's reserved zero row in each chunk

    per_core = []
    for c in range(NC):
        sel, cnt = cores[c]
        starts = np.concatenate([[0], np.cumsum(cnt.reshape(-1))])
        idx_flat = np.zeros(TT * 128, np.int16)
        col_flat = np.full(TT * 128, PAD_COL, np.float32)
        for w in range(NW):
            for k in range(NKP):
                cell = w * NKP + k
                off = int(tile_base[w, k]) * 128
                ntile = int(T[w, k])
                idx_flat[off:off + ntile * 128] = pad_idx_k[k]
                n = int(cnt[w, k])
                if n == 0:
                    continue
                e = sel[starts[cell]:starts[cell] + n]
                # ascending src order within the cell: consecutive gather
                # descriptors hit nearby HBM addresses (row-buffer locality)
                e = e[np.argsort(idx_e[e], kind="stable")]
                idx_flat[off:off + n] = idx_e[e]
                col_flat[off:off + n] = col_e[e].astype(np.float32)

        idx_arr = np.zeros((128, IDX_COLS), np.int16)
        for cgi, cgrp in enumerate(CGROUPS):
            for k in range(NKP):
                o = call_off[(cgi, k)]
                n = call_n[(cgi, k)]
                fo = int(tile_base[cgrp[0], k]) * 128
                block = idx_flat[fo:fo + n].reshape(-1, 16).T  # [16, n/16]
                idx_arr[:, o:o + n // 16] = np.tile(block, (8, 1))

        colid = np.ascontiguousarray(col_flat.reshape(TT, 128).T)  # [128, TT]

        # node-ordered shard arrays (row n_local = w*128 + slot)
        pc = perms[c]

        def shardify(full, width):
            arr = np.zeros((SH, width), np.float32)
            arr[pc] = full[c * SR:(c + 1) * SR]
            return np.ascontiguousarray(
                arr.reshape(NW, 128, width).transpose(1, 0, 2)
                .reshape(128, NW * width))

        x_c = shardify(x, DIM)
        nr_c = shardify(nr, DIM)
        a_pad = np.zeros(SH, np.float32); a_pad[pc] = a[c * SR:(c + 1) * SR]
        a2_pad = np.zeros(SH, np.float32); a2_pad[pc] = a2[c * SR:(c + 1) * SR]
        per_core.append(dict(
            idx=idx_arr, colid=colid, x=x_c, nr=nr_c, perm=pc,
            a=np.ascontiguousarray(a_pad.reshape(NW, 128).T),
            a2=np.ascontiguousarray(a2_pad.reshape(NW, 128).T),
        ))

    consts = dict(
        iota=np.tile(np.arange(128, dtype=np.float16), (128, 1)),
        ident=np.eye(128, dtype=np.float32),
        w1=np.ascontiguousarray(W1),                                   # [128,256]
        w2=np.ascontiguousarray(np.concatenate([W2[0:128], W2[128:256]], axis=1)),  # [128,256]
        b1=np.ascontiguousarray(b1.reshape(2, 128).T),                 # [128,2]
        b2=np.ascontiguousarray(b2.reshape(128, 1)),                   # [128,1]
    )
    struct = dict(T=T, TT=TT, IDX_COLS=IDX_COLS, call_off=call_off,
                  call_n=call_n, tile_base=tile_base, att=att)
    return per_core, consts, struct


def _build(struct):
    import concourse.bacc as bacc
    import concourse.bass as bass
    import concourse.mybir as mybir
    import concourse.tile as tile
    from concourse.library_config import mlp

    f32 = mybir.dt.float32
    f16 = mybir.dt.float16
    i16 = mybir.dt.int16
    EQ = mybir.AluOpType.is_equal
    MUL = mybir.AluOpType.mult
    ADD = mybir.AluOpType.add

    T = struct["T"]; TT = struct["TT"]; IDX_COLS = struct["IDX_COLS"]
    call_off = struct["call_off"]; call_n = struct["call_n"]
    tile_base = struct["tile_base"]; att = struct["att"]
    nw_k = [w1 - w0 for (w0, w1) in PIECES]

    nc = bacc.Bacc("TRN2", target_bir_lowering=False, debug=False,
                   num_devices=NC, num_swdge_queues=4)

    x_in = nc.dram_tensor("x_in", [128, NW * DIM], f32, kind="ExternalInput")
    nr_in = nc.dram_tensor("nr_in", [128, NW * DIM], f32, kind="ExternalInput")
    idx_in = nc.dram_tensor("idx_in", [128, IDX_COLS], i16, kind="ExternalInput")
    colid_in = nc.dram_tensor("colid_in", [128, TT], f32, kind="ExternalInput")
    a_in = nc.dram_tensor("a_in", [128, NW], f32, kind="ExternalInput")
    a2_in = nc.dram_tensor("a2_in", [128, NW], f32, kind="ExternalInput")
    iota_in = nc.dram_tensor("iota_in", [128, 128], f16, kind="ExternalInput")
    ident_in = nc.dram_tensor("ident_in", [128, 128], f32, kind="ExternalInput")
    w1_in = nc.dram_tensor("w1_in", [128, 256], f32, kind="ExternalInput")
    w2_in = nc.dram_tensor("w2_in", [128, 256], f32, kind="ExternalInput")
    b1_in = nc.dram_tensor("b1_in", [128, 2], f32, kind="ExternalInput")
    b2_in = nc.dram_tensor("b2_in", [128, 1], f32, kind="ExternalInput")
    out_msg = nc.dram_tensor("out_msg", [128, NW * DIM], f32, kind="ExternalOutput")

    with tile.TileContext(nc) as tc:
        nc.gpsimd.load_library(mlp)
        with (
            tc.tile_pool(name="const", bufs=1) as cpool,
            tc.tile_pool(name="dram", bufs=1, space="DRAM") as dpool,
            tc.tile_pool(name="msg", bufs=_MPOOL) as mpool,
            tc.tile_pool(name="s", bufs=48) as spool,
            tc.tile_pool(name="io", bufs=4) as iopool,
            tc.tile_pool(name="work", bufs=2) as wpool,
        ):
            # persistent SBUF state
            idx_sb = cpool.tile([128, IDX_COLS], i16, tag="idx")
            colid_sb = cpool.tile([128, TT], f32, tag="colid")
            a_sb = cpool.tile([128, NW], f32, tag="a")
            a2_sb = cpool.tile([128, NW], f32, tag="a2")
            iota_sb = cpool.tile([128, 128], f16, tag="iota")
            ident_sb = cpool.tile([128, 128], f32, tag="ident")
            w1_sb = cpool.tile([128, 256], f32, tag="w1")
            w2_sb = cpool.tile([128, 256], f32, tag="w2")
            b1_sb = cpool.tile([128, 2], f32, tag="b1")
            b2_sb = cpool.tile([128, 1], f32, tag="b2")
            stage = cpool.tile([128, NW, 128], f16, tag="stage")
            rho = cpool.tile([128, NW, 128], f16 if _RHO16 else f32, tag="rho")

            for sb, dr in [(idx_sb, idx_in), (colid_sb, colid_in),
                           (a_sb, a_in), (a2_sb, a2_in), (iota_sb, iota_in),
                           (ident_sb, ident_in), (w1_sb, w1_in), (w2_sb, w2_in),
                           (b1_sb, b1_in), (b2_sb, b2_in)]:
                nc.sync.dma_start(sb[:], dr[:])

            nc.vector.memset(rho[:], 0.0)

            cc_in = []
            cc_out = []
            for i in range(HOPS):
                cc_in.append(dpool.tile([SH, DIM], f16, tag=f"cc_in{i % 2}",
                                        name=f"cc_in{i}"))
                cc_out.append([
                    dpool.tile([NC * nw_k[j] * 128, DIM], f16,
                               tag=f"cc_out{i}_{j}", name=f"cc_out{i}_{j}",
                               addr_space="Shared")
                    for j in range(NKP)])

            def issue_ag(t, j):
                w0, w1 = PIECES[j]
                if _SKIP_COLL:
                    nc.sync.dma_start(cc_out[t][j][0:(w1 - w0) * 128, :],
                                      cc_in[t][w0 * 128:w1 * 128, :])
                else:
                    nc.gpsimd.collective_compute(
                        "AllGather", mybir.AluOpType.bypass,
                        replica_groups=[list(range(NC))],
                        ins=[cc_in[t][w0 * 128:w1 * 128, :]],
                        outs=[cc_out[t][j][:]])

            def bundle_ops(out_ap, nr_ap, in_ap, tmp_ap, inverse):
                """out[b,c,e] = sum_d nr[b, c, d or (d,c)] * in[b,d,e] on DVE."""
                nr4 = nr_ap.rearrange("p (b c d) -> p b c d", b=8, c=4, d=4)
                in4 = in_ap.rearrange("p (b d e) -> p b d e", b=8, d=4, e=4)
                out4 = out_ap.rearrange("p (b c e) -> p b c e", b=8, c=4, e=4)
                tmp4 = tmp_ap.rearrange("p (b c e) -> p b c e", b=8, c=4, e=4)
                for d in range(4):
                    if inverse:
                        nr_d = nr4[:, :, d:d + 1, :].rearrange("p b o c -> p b (o c)")
                        nr_b = nr_d.unsqueeze(3).broadcast_to((128, 8, 4, 4))
                    else:
                        nr_b = nr4[:, :, :, d:d + 1].broadcast_to((128, 8, 4, 4))
                    in_b = in4[:, :, d:d + 1, :].broadcast_to((128, 8, 4, 4))
                    tgt = out4 if d == 0 else tmp4
                    nc.any.tensor_tensor(tgt, nr_b, in_b, MUL)
                    if d > 0:
                        nc.any.tensor_tensor(out4, out4, tmp4, ADD)

            # cc_in viewed as [slot partition, window, feature]
            def cc_in_view(t):
                return cc_in[t][:].rearrange("(w p) e -> p w e", w=NW, p=128)

            # ---- pre-stage: h0 = bundle(nr, x); stage = a * h0 (fp16) ----
            for w in range(NW):
                x_t = iopool.tile([128, 128], f32, tag="xt")
                nr_t = iopool.tile([128, 128], f32, tag="nrt")
                nc.sync.dma_start(x_t[:], x_in[:, w * 128:(w + 1) * 128])
                nc.sync.dma_start(nr_t[:], nr_in[:, w * 128:(w + 1) * 128])
                h0 = wpool.tile([128, 128], f32, tag="h0")
                tmp = wpool.tile([128, 128], f32, tag="tmpb")
                bundle_ops(h0[:], nr_t[:], x_t[:], tmp[:], inverse=False)
                nc.any.tensor_scalar(stage[:, w, :], h0[:], a_sb[:, w:w + 1],
                                     None, MUL)
                nc.sync.dma_start(cc_in_view(0)[:, w:w + 1, :],
                                  stage[:, w:w + 1, :])
                for j in range(NKP):
                    if w == PIECES[j][1] - 1:
                        issue_ag(0, j)

            # ---- hop loop ----
            with tc.tile_pool(name="psum", bufs=2, space="PSUM") as pspool:
                for t in range(1, HOPS + 1):
                    prev = cc_out[t - 1]
                    for cgi, cgrp in enumerate(CGROUPS):
                        msgs = {}
                        for k in range(NKP):
                            n = call_n[(cgi, k)]
                            o = call_off[(cgi, k)]
                            ntile = n // 128
                            msg = mpool.tile([128, ntile, 128], f16, tag="msg",
                                             name="msg")
                            msgs[k] = msg
                            if _SKIP_GATHER:
                                nc.vector.memset(msg[:], 0.0)
                            else:
                                nc.gpsimd.dma_gather(
                                    msg[:], prev[k][:, :],
                                    idx_sb[:, o:o + n // 16], n, n, 128,
                                    single_packet=False,
                                    queue_num=(cgi + k) % 4)
                        for sub in range(0, len(cgrp), GRP):
                            grp = cgrp[sub:sub + GRP]
                            psums = {w: pspool.tile([128, 128], f32,
                                                    tag=f"ps{i}", name=f"ps{i}")
                                     for i, w in enumerate(grp)}
                            # build every S tile of the sub-group before its
                            # matmuls so DVE/ACT pacing never gates PE
                            s_tiles = {}
                            for k in range(NKP):
                                for w in grp:
                                    for j in range(int(T[w, k])):
                                        tix = int(tile_base[w, k]) + j
                                        s_t = spool.tile([128, 128], f16,
                                                         tag="s", name="s_t")
                                        nc.any.tensor_scalar(
                                            s_t[:], iota_sb[:],
                                            colid_sb[:, tix:tix + 1], None, EQ)
                                        s_tiles[(k, w, j)] = s_t
                            for k in range(NKP):
                                for w in grp:
                                    for j in range(int(T[w, k])):
                                        tix = int(tile_base[w, k]) + j
                                        jj = tix - int(tile_base[cgrp[0], k])
                                        nc.tensor.matmul(
                                            psums[w][:], s_tiles[(k, w, j)][:],
                                            msgs[k][:, jj, :],
                                            start=(k == 0 and j == 0),
                                            stop=(k == NKP - 1
                                                  and j == int(T[w, NKP - 1]) - 1))
                            for w in grp:
                                if t in SNAPS:
                                    tmp = wpool.tile([128, 128],
                                                     f16 if _RHO16 else f32,
                                                     tag="snap")
                                    nc.any.tensor_scalar(tmp[:], psums[w][:],
                                                         float(att[SNAPS[t]]),
                                                         None, MUL)
                                    nc.any.tensor_tensor(rho[:, w, :],
                                                         rho[:, w, :],
                                                         tmp[:], ADD)
                                if t < HOPS:
                                    nc.vector.tensor_scalar(stage[:, w, :],
                                                            psums[w][:],
                                                            a2_sb[:, w:w + 1],
                                                            None, MUL)
                            if t < HOPS:
                                g0 = grp[0]
                                nc.sync.dma_start(
                                    cc_in_view(t)[:, g0:g0 + len(grp), :],
                                    stage[:, g0:g0 + len(grp), :])
                        if t < HOPS and cgi in AG_AT:
                            for j in AG_AT[cgi]:
                                issue_ag(t, j)

            # ---- post: r = a*rho; FFN; inverse bundle; write out ----
            with tc.tile_pool(name="psum2", bufs=1, space="PSUM") as ps2pool:
                for c0 in range(0, NW, 4):
                    ws = list(range(c0, min(c0 + 4, NW)))
                    nwc = len(ws)
                    rT = wpool.tile([128, 4 * 128], f32, tag="rT")
                    for i, w in enumerate(ws):
                        r_t = wpool.tile([128, 128], f32, tag="rt")
                        nc.any.tensor_scalar(r_t[:], rho[:, w, :],
                                             a_sb[:, w:w + 1], None, MUL)
                        p_t = ps2pool.tile([128, 128], f32, tag="ptr")
                        nc.tensor.transpose(p_t[:], r_t[:], ident_sb[:])
                        nc.vector.tensor_copy(rT[:, i * 128:(i + 1) * 128],
                                              p_t[:])
                    nn = nwc * 128
                    pg0 = ps2pool.tile([128, 512], f32, tag="pg0")
                    pg1 = ps2pool.tile([128, 512], f32, tag="pg1")
                    nc.tensor.matmul(pg0[:, :nn], w1_sb[:, 0:128], rT[:, :nn],
                                     start=True, stop=True)
                    nc.tensor.matmul(pg1[:, :nn], w1_sb[:, 128:256], rT[:, :nn],
                                     start=True, stop=True)
                    g0_sb = wpool.tile([128, 4 * 128], f32, tag="g0")
                    g1_sb = wpool.tile([128, 4 * 128], f32, tag="g1")
                    nc.scalar.activation(g0_sb[:, :nn], pg0[:, :nn],
                                         mybir.ActivationFunctionType.Gelu,
                                         bias=b1_sb[:, 0:1])
                    nc.scalar.activation(g1_sb[:, :nn], pg1[:, :nn],
                                         mybir.ActivationFunctionType.Gelu,
                                         bias=b1_sb[:, 1:2])
                    ph = ps2pool.tile([128, 512], f32, tag="ph")
                    nc.tensor.matmul(ph[:, :nn], w2_sb[:, 0:128], g0_sb[:, :nn],
                                     start=True, stop=False)
                    nc.tensor.matmul(ph[:, :nn], w2_sb[:, 128:256],
                                     g1_sb[:, :nn], start=False, stop=True)
                    h2 = wpool.tile([128, 4 * 128], f32, tag="h2")
                    nc.any.tensor_scalar(h2[:, :nn], ph[:, :nn], b2_sb[:, 0:1],
                                         None, ADD)
                    for i, w in enumerate(ws):
                        pb = ps2pool.tile([128, 128], f32, tag="pb")
                        nc.tensor.transpose(pb[:], h2[:, i * 128:(i + 1) * 128],
                                            ident_sb[:])
                        hb = wpool.tile([128, 128], f32, tag="hb")
                        nc.vector.tensor_copy(hb[:], pb[:])
                        nr_t = iopool.tile([128, 128], f32, tag="nrt2")
                        nc.sync.dma_start(nr_t[:],
                                          nr_in[:, w * 128:(w + 1) * 128])
                        mo = wpool.tile([128, 128], f32, tag="mo")
                        tmp = wpool.tile([128, 128], f32, tag="tmpb2")
                        bundle_ops(mo[:], nr_t[:], hb[:], tmp[:], inverse=True)
                        nc.sync.dma_start(out_msg[:, w * 128:(w + 1) * 128],
                                          mo[:])

    nc.compile()
    return nc


LAST_RESULTS = None  # BassKernelResults of the most recent kernel() call
LAST_NC = None
LAST_IN_MAPS = None


def kernel(**inputs) -> np.ndarray:
    from concourse.bass_utils import run_bass_kernel_spmd

    per_core, consts, struct = _prep_cached(inputs)
    nc = _build(struct)

    in_maps = []
    for c in range(NC):
        d = per_core[c]
        in_maps.append({
            "x_in": d["x"], "nr_in": d["nr"], "idx_in": d["idx"],
            "colid_in": d["colid"], "a_in": d["a"], "a2_in": d["a2"],
            "iota_in": consts["iota"], "ident_in": consts["ident"],
            "w1_in": consts["w1"], "w2_in": consts["w2"],
            "b1_in": consts["b1"], "b2_in": consts["b2"],
        })

    trace = __import__("os").environ.get("KTRACE", "") == "1"
    res = run_bass_kernel_spmd(nc, in_maps, core_ids=list(range(NC)),
                               trace=trace)
    global LAST_RESULTS, LAST_NC, LAST_IN_MAPS
    LAST_RESULTS = res
    LAST_NC = nc
    LAST_IN_MAPS = in_maps

    x = np.asarray(inputs["x"], np.float32)
    msg = np.empty((N, DIM), np.float32)
    for c in range(NC):
        arr = res.results[c]["out_msg"].reshape(128, NW, DIM)
        arr = np.ascontiguousarray(arr.transpose(1, 0, 2)).reshape(SH, DIM)
        msg[c * SR:(c + 1) * SR] = arr[per_core[c]["perm"]]
    return np.concatenate([x, msg], axis=1)
